# revision 20
# baseline (speedup 1.0000x reference)
"""Committee-vote histogram kernel for TRN2 (8 NeuronCores, data-parallel).

votes[b, c] = sum_m 1[argmax_c' (x[b] @ W[m, :, c'] + b[m, c']) == c]

Strategy per core (batch shard of 8192 rows):
  - x is decomposed host-side into an exact fp16 pair (x = xh + xl with
    residual ~2^-22|x|); likewise W and the bias. Logits are computed as
    xh@Wh + xh@Wl + xl@Wh (+bias), whose decomposition error (~2e-7) is at
    fp32 rounding level — validated exact-match against the fp32 reference.
  - The halves are stored host-side in [d', k, b] layout so loads are plain
    dense DMAs; 8 chunks of 1024 rows stream on the two HWDGE rings
    (sync: xh, scalar: xl), all dispatched upfront — each ring carries only
    loads, so ring-space waits at the queue head block nothing.
  - Bias is added by seeding each PSUM accumulation group with a K=2
    matmul of ones against the replicated (bh|bl) rows.
  - Filler matmuls (never read) keep the PE HAM activity window busy so the
    clock gate stays at 2.4 GHz; with the periodic DMA-wait gaps the PE
    would otherwise stay at 1.2 GHz and become the kernel bottleneck.
  - Votes per 4-tile batch: DVE reduce_max / is_ge straight from PSUM, then
    a 3-level all-bf16 contiguous add tree (2x DVE mode) sums the 8 member
    masks. bf16 is exact here (0/1 masks, counts <= 8). Results accumulate
    in one bf16 staging tile, stored once at the end; host unscrambles.
"""

import os
import sys

import numpy as np

if os.path.isdir("/opt/trn_rl_repo") and "/opt/trn_rl_repo" not in sys.path:
    sys.path.insert(0, "/opt/trn_rl_repo")

import concourse.bass as bass
import concourse.tile as tile
from concourse import bacc, mybir
from concourse.bass import ts

F32 = mybir.dt.float32
F16 = mybir.dt.float16
BF16 = mybir.dt.bfloat16

B_FULL = 65536
D = 256
C = 10
M = 8
N_CORES = 8
B_SHARD = B_FULL // N_CORES  # 8192
P = 128

MC = M * C  # 80 logit columns per sample
CHUNK = 1024  # batch rows per chunk (512KB per HWDGE ring per chunk)
WARMUP_MM = 14  # pre-stream filler matmuls: ramp the PE HAM clock gate
FILLER_MM = 10  # per-chunk fillers: keep every HAM window busy (stay warm)


def build_nc(b_shard: int = B_SHARD) -> bass.Bass:
    chunk = min(CHUNK, b_shard)
    n_chunks = b_shard // chunk
    assert b_shard % chunk == 0
    tiles_per_chunk = chunk // P
    batches_per_chunk = tiles_per_chunk // 4  # vote batch = 4 tiles
    assert batches_per_chunk * 4 == tiles_per_chunk
    n_batches = n_chunks * batches_per_chunk

    nc = bacc.Bacc("TRN2", target_bir_lowering=False)
    # x halves in [d', k, b] layout (d = 128k + d'), prepared host-side
    xh = nc.dram_tensor("xh", [P, 2, b_shard], F16, kind="ExternalInput")
    xl = nc.dram_tensor("xl", [P, 2, b_shard], F16, kind="ExternalInput")
    wh = nc.dram_tensor("wh", [D, MC], F16, kind="ExternalInput")
    wl = nc.dram_tensor("wl", [D, MC], F16, kind="ExternalInput")
    bc4 = nc.dram_tensor("bc4", [2, 4 * MC], F16, kind="ExternalInput")
    # staging: ys[p, b*40 + t*C + c] = votes[b*512 + t*128 + p, c]
    ys = nc.dram_tensor("ys", [P, n_batches * 4 * C], BF16,
                        kind="ExternalOutput")

    with tile.TileContext(nc) as tc:
        with (
            tc.tile_pool(name="consts", bufs=1) as consts,
            tc.tile_pool(name="xt", bufs=1) as xt_pool,
            tc.tile_pool(name="scr", bufs=1, space="PSUM") as scr_pool,
            tc.tile_pool(name="lg", bufs=6, space="PSUM") as lg_pool,
            tc.tile_pool(name="mx", bufs=3) as mx_pool,
            tc.tile_pool(name="eq", bufs=3) as eq_pool,
            tc.tile_pool(name="t4", bufs=3) as t4_pool,
            tc.tile_pool(name="t2", bufs=3) as t2_pool,
            tc.tile_pool(name="stg", bufs=1) as stg_pool,
        ):
            # W halves as [128 d', k, 80] where d = 128k + d'
            wh_sb = consts.tile([P, 2, MC], F16)
            nc.sync.dma_start(wh_sb, wh.rearrange("(k p) c -> p k c", p=P))
            wl_sb = consts.tile([P, 2, MC], F16)
            nc.scalar.dma_start(wl_sb, wl.rearrange("(k p) c -> p k c", p=P))
            bc4_sb = consts.tile([2, 4 * MC], F16)
            nc.scalar.dma_start(bc4_sb, bc4[:])
            ones2 = consts.tile([2, P], F16)
            nc.vector.memset(ones2, 1.0)

            # slot layout per chunk: [128 d', (hl k), chunk b]
            # slot 0: xh k=0, 1: xh k=1, 2: xl k=0, 3: xl k=1
            xt = [
                xt_pool.tile([P, 4, chunk], F16, name=f"xt{g}")
                for g in range(n_chunks)
            ]
            for g in range(n_chunks):
                sl = np.s_[:, :, g * chunk : (g + 1) * chunk]
                nc.sync.dma_start(xt[g][:, 0:2, :], xh[sl])
                nc.scalar.dma_start(xt[g][:, 2:4, :], xl[sl])

            # votes accumulate here; stored once at the end
            stg = stg_pool.tile([P, n_batches * 4 * C], BF16)

            # PE filler target: written, never read
            scr = scr_pool.tile([P, 4 * MC], F32)

            def filler(n):
                for _ in range(n):
                    nc.tensor.matmul(
                        scr, lhsT=ones2, rhs=bc4_sb, start=True, stop=True,
                    )

            def votes(b, lg):
                mx = mx_pool.tile([P, 4 * M], F32, name="mx")
                nc.vector.reduce_max(
                    mx,
                    lg[:].rearrange("p (a c) -> p a c", c=C),
                    axis=mybir.AxisListType.X,
                )
                # mask written (t, m, c)-ordered: every DVE operand below is
                # unit-stride/contiguous (strided DVE writes run ~3x slower)
                eq = eq_pool.tile([P, 4 * MC], BF16, name="eq")
                nc.vector.tensor_tensor(
                    out=eq[:].rearrange("p (t m c) -> p t m c", t=4, m=M, c=C),
                    in0=lg[:].rearrange("p (t m c) -> p t m c", t=4, m=M, c=C),
                    in1=mx[:, :, None]
                    .rearrange("p (t m) c -> p t m c", t=4)
                    .broadcast_to([P, 4, M, C]),
                    op=mybir.AluOpType.is_ge,
                )
                # member-sum over m: 3-level all-bf16 contiguous add tree
                # (2x DVE mode); bf16 is exact for 0/1 masks and counts <= 8
                t4 = t4_pool.tile([P, 4 * 4 * C], BF16, name="t4")
                eqv = eq[:].rearrange("p (t m c) -> p t m c", t=4, m=M, c=C)
                nc.vector.tensor_tensor(
                    out=t4[:].rearrange("p (t m c) -> p t m c", t=4, m=4, c=C),
                    in0=eqv[:, :, 0:4, :],
                    in1=eqv[:, :, 4:8, :],
                    op=mybir.AluOpType.add,
                )
                t2 = t2_pool.tile([P, 4 * 2 * C], BF16, name="t2")
                t4v = t4[:].rearrange("p (t m c) -> p t m c", t=4, m=4, c=C)
                nc.vector.tensor_tensor(
                    out=t2[:].rearrange("p (t m c) -> p t m c", t=4, m=2, c=C),
                    in0=t4v[:, :, 0:2, :],
                    in1=t4v[:, :, 2:4, :],
                    op=mybir.AluOpType.add,
                )
                t2v = t2[:].rearrange("p (t m c) -> p t m c", t=4, m=2, c=C)
                nc.vector.tensor_tensor(
                    out=stg[:, ts(b, 4 * C)].rearrange("p (t c) -> p t c", c=C),
                    in0=t2v[:, :, 0, :],
                    in1=t2v[:, :, 1, :],
                    op=mybir.AluOpType.add,
                )

            filler(WARMUP_MM)

            for g in range(n_chunks):
                for bi in range(batches_per_chunk):
                    # logits for this 4-tile batch, bias-seeded
                    lg = lg_pool.tile([P, 4 * MC], F32, name="lg")
                    nc.tensor.matmul(
                        lg, lhsT=ones2, rhs=bc4_sb, start=True, stop=False
                    )
                    for j in range(4):
                        t = bi * 4 + j
                        for k in range(2):
                            xh_c = xt[g][:, k, ts(t, P)]
                            xl_c = xt[g][:, 2 + k, ts(t, P)]
                            o = lg[:, ts(j, MC)]
                            nc.tensor.matmul(
                                o, lhsT=xh_c, rhs=wh_sb[:, k, :],
                                start=False, stop=False,
                            )
                            nc.tensor.matmul(
                                o, lhsT=xh_c, rhs=wl_sb[:, k, :],
                                start=False, stop=False,
                            )
                            nc.tensor.matmul(
                                o, lhsT=xl_c, rhs=wh_sb[:, k, :],
                                start=False, stop=(j == 3 and k == 1),
                            )
                    votes(g * batches_per_chunk + bi, lg)
                filler(FILLER_MM)

            # single store at the end: the sync ring is idle once loads drain
            nc.sync.dma_start(ys[:], stg[:])
    nc.compile()
    return nc


_NC_CACHE: dict[int, bass.Bass] = {}


def _get_nc(b_shard: int) -> bass.Bass:
    if b_shard not in _NC_CACHE:
        _NC_CACHE[b_shard] = build_nc(b_shard)
    return _NC_CACHE[b_shard]


def _prep_inputs(x: np.ndarray, W: np.ndarray, b: np.ndarray):
    xf = np.asarray(x, dtype=np.float32)
    xh = xf.astype(np.float16)
    xl = (xf - xh.astype(np.float32)).astype(np.float16)
    # [B, 256] -> [128 d', 2 k, B] with d = 128k + d'
    parts = {
        "xh": xh.T.reshape(2, P, B_FULL).transpose(1, 0, 2),
        "xl": xl.T.reshape(2, P, B_FULL).transpose(1, 0, 2),
    }
    # m-major columns: col index = 10*m + c
    wf = np.asarray(W, dtype=np.float32).transpose(1, 0, 2).reshape(D, MC)
    whf = wf.astype(np.float16)
    wlf = (wf - whf.astype(np.float32)).astype(np.float16)
    bf = np.asarray(b, dtype=np.float32).reshape(MC)
    bh = bf.astype(np.float16)
    bl = (bf - bh.astype(np.float32)).astype(np.float16)
    bc4 = np.ascontiguousarray(
        np.stack([np.tile(bh, 4), np.tile(bl, 4)], axis=0)
    ).astype(np.float16)
    return parts, np.ascontiguousarray(whf), np.ascontiguousarray(wlf), bc4


def _unscramble(ys: np.ndarray) -> np.ndarray:
    # ys[p, b*40 + t*C + c] -> votes[b*512 + t*128 + p, c]
    n_batches = ys.shape[1] // (4 * C)
    return np.ascontiguousarray(
        ys.astype(np.float32)
        .reshape(P, n_batches, 4, C)
        .transpose(1, 2, 0, 3)
        .reshape(n_batches * 4 * P, C)
    )


def kernel(x: np.ndarray, W: np.ndarray, b: np.ndarray, **_) -> np.ndarray:
    from concourse.bass_utils import run_bass_kernel_spmd

    assert x.shape == (B_FULL, D), x.shape
    parts, whf, wlf, bc4 = _prep_inputs(x, W, b)

    nc = _get_nc(B_SHARD)
    in_maps = [
        {
            **{
                k: np.ascontiguousarray(v[:, :, i * B_SHARD : (i + 1) * B_SHARD])
                for k, v in parts.items()
            },
            "wh": whf,
            "wl": wlf,
            "bc4": bc4,
        }
        for i in range(N_CORES)
    ]
    res = run_bass_kernel_spmd(nc, in_maps, core_ids=list(range(N_CORES)))
    return np.concatenate(
        [_unscramble(res.results[i]["ys"]) for i in range(N_CORES)], axis=0
    )


# revision 24
# speedup vs baseline: 1.6403x; 1.6403x over previous
"""Committee-vote histogram kernel for TRN2 (8 NeuronCores, data-parallel).

votes[b, c] = sum_m 1[argmax_c' (x[b] @ W[m, :, c'] + b[m, c']) == c]

Strategy per core (batch shard of 8192 rows):
  - x is decomposed host-side into an exact fp16 pair (x = xh + xl with
    residual ~2^-22|x|); likewise W and the bias. Logits are computed as
    xh@Wh + xh@Wl + xl@Wh (+bias), whose decomposition error (~2e-7) is at
    fp32 rounding level — validated exact-match against the fp32 reference.
  - The halves are stored host-side in [d', k, b] layout so loads are plain
    dense DMAs; 8 chunks of 1024 rows stream on the two HWDGE rings
    (sync: xh, scalar: xl), all dispatched upfront — each ring carries only
    loads, so ring-space waits at the queue head block nothing.
  - Bias is added by seeding each PSUM accumulation group with a K=2
    matmul of ones against the replicated (bh|bl) rows.
  - Filler matmuls (never read) keep the PE HAM activity window busy so the
    clock gate stays at 2.4 GHz; with the periodic DMA-wait gaps the PE
    would otherwise stay at 1.2 GHz and become the kernel bottleneck.
  - Votes per 4-tile batch: DVE reduce_max / is_ge straight from PSUM, then
    a 3-level all-bf16 contiguous add tree (2x DVE mode) sums the 8 member
    masks. bf16 is exact here (0/1 masks, counts <= 8). Results accumulate
    in one bf16 staging tile, stored once at the end; host unscrambles.
"""

import os
import sys

import numpy as np

if os.path.isdir("/opt/trn_rl_repo") and "/opt/trn_rl_repo" not in sys.path:
    sys.path.insert(0, "/opt/trn_rl_repo")

import concourse.bass as bass
import concourse.tile as tile
from concourse import bacc, mybir
from concourse.bass import ts

F32 = mybir.dt.float32
F16 = mybir.dt.float16
BF16 = mybir.dt.bfloat16

B_FULL = 65536
D = 256
C = 10
M = 8
N_CORES = 8
B_SHARD = B_FULL // N_CORES  # 8192
P = 128

MC = M * C  # 80 logit columns per sample
CHUNK = 1024  # batch rows per chunk (512KB per HWDGE ring per chunk)
PE_DELAY_CHUNKS = 2  # gate the PE until this chunk arrived: the backlog keeps
# the PE gap-free afterwards, so the HAM clock gate warms to 2.4 GHz and stays
# there (chunk-by-chunk processing leaves a gap in every 3.4us HAM window and
# pins the PE at 1.2 GHz, making it the kernel bottleneck)


def build_nc(b_shard: int = B_SHARD) -> bass.Bass:
    chunk = min(CHUNK, b_shard)
    n_chunks = b_shard // chunk
    assert b_shard % chunk == 0
    tiles_per_chunk = chunk // P
    batches_per_chunk = tiles_per_chunk // 4  # vote batch = 4 tiles
    assert batches_per_chunk * 4 == tiles_per_chunk
    n_batches = n_chunks * batches_per_chunk

    nc = bacc.Bacc("TRN2", target_bir_lowering=False)
    # x halves in [d', k, b] layout (d = 128k + d'), prepared host-side
    xh = nc.dram_tensor("xh", [P, 2, b_shard], F16, kind="ExternalInput")
    xl = nc.dram_tensor("xl", [P, 2, b_shard], F16, kind="ExternalInput")
    wh = nc.dram_tensor("wh", [D, MC], F16, kind="ExternalInput")
    wl = nc.dram_tensor("wl", [D, MC], F16, kind="ExternalInput")
    bc4 = nc.dram_tensor("bc4", [2, 4 * MC], F16, kind="ExternalInput")
    # staging: ys[p, b*40 + t*C + c] = votes[b*512 + t*128 + p, c]
    ys = nc.dram_tensor("ys", [P, n_batches * 4 * C], BF16,
                        kind="ExternalOutput")

    with tile.TileContext(nc) as tc:
        with (
            tc.tile_pool(name="consts", bufs=1) as consts,
            tc.tile_pool(name="xt", bufs=1) as xt_pool,
            tc.tile_pool(name="scr", bufs=1, space="PSUM") as scr_pool,
            tc.tile_pool(name="lg", bufs=6, space="PSUM") as lg_pool,
            tc.tile_pool(name="mx", bufs=3) as mx_pool,
            tc.tile_pool(name="eq", bufs=3) as eq_pool,
            tc.tile_pool(name="t4", bufs=3) as t4_pool,
            tc.tile_pool(name="t2", bufs=3) as t2_pool,
            tc.tile_pool(name="stg", bufs=1) as stg_pool,
        ):
            # W halves as [128 d', k, 80] where d = 128k + d'
            wh_sb = consts.tile([P, 2, MC], F16)
            nc.sync.dma_start(wh_sb, wh.rearrange("(k p) c -> p k c", p=P))
            wl_sb = consts.tile([P, 2, MC], F16)
            nc.scalar.dma_start(wl_sb, wl.rearrange("(k p) c -> p k c", p=P))
            bc4_sb = consts.tile([2, 4 * MC], F16)
            nc.scalar.dma_start(bc4_sb, bc4[:])
            ones2 = consts.tile([2, P], F16)
            nc.vector.memset(ones2, 1.0)

            # slot layout per chunk: [128 d', (hl k), chunk b]
            # slot 0: xh k=0, 1: xh k=1, 2: xl k=0, 3: xl k=1
            xt = [
                xt_pool.tile([P, 4, chunk], F16, name=f"xt{g}")
                for g in range(n_chunks)
            ]
            for g in range(n_chunks):
                sl = np.s_[:, :, g * chunk : (g + 1) * chunk]
                nc.sync.dma_start(xt[g][:, 0:2, :], xh[sl])
                nc.scalar.dma_start(xt[g][:, 2:4, :], xl[sl])

            # votes accumulate here; stored once at the end
            stg = stg_pool.tile([P, n_batches * 4 * C], BF16)

            # PE delay-gate target: written, never read
            scr = scr_pool.tile([P, MC], F32)

            def votes(b, lg):
                mx = mx_pool.tile([P, 4 * M], F32, name="mx")
                nc.vector.reduce_max(
                    mx,
                    lg[:].rearrange("p (a c) -> p a c", c=C),
                    axis=mybir.AxisListType.X,
                )
                # mask written (t, m, c)-ordered: every DVE operand below is
                # unit-stride/contiguous (strided DVE writes run ~3x slower)
                eq = eq_pool.tile([P, 4 * MC], BF16, name="eq")
                nc.vector.tensor_tensor(
                    out=eq[:].rearrange("p (t m c) -> p t m c", t=4, m=M, c=C),
                    in0=lg[:].rearrange("p (t m c) -> p t m c", t=4, m=M, c=C),
                    in1=mx[:, :, None]
                    .rearrange("p (t m) c -> p t m c", t=4)
                    .broadcast_to([P, 4, M, C]),
                    op=mybir.AluOpType.is_ge,
                )
                # member-sum over m: 3-level all-bf16 contiguous add tree
                # (2x DVE mode); bf16 is exact for 0/1 masks and counts <= 8
                t4 = t4_pool.tile([P, 4 * 4 * C], BF16, name="t4")
                eqv = eq[:].rearrange("p (t m c) -> p t m c", t=4, m=M, c=C)
                nc.vector.tensor_tensor(
                    out=t4[:].rearrange("p (t m c) -> p t m c", t=4, m=4, c=C),
                    in0=eqv[:, :, 0:4, :],
                    in1=eqv[:, :, 4:8, :],
                    op=mybir.AluOpType.add,
                )
                t2 = t2_pool.tile([P, 4 * 2 * C], BF16, name="t2")
                t4v = t4[:].rearrange("p (t m c) -> p t m c", t=4, m=4, c=C)
                nc.vector.tensor_tensor(
                    out=t2[:].rearrange("p (t m c) -> p t m c", t=4, m=2, c=C),
                    in0=t4v[:, :, 0:2, :],
                    in1=t4v[:, :, 2:4, :],
                    op=mybir.AluOpType.add,
                )
                t2v = t2[:].rearrange("p (t m c) -> p t m c", t=4, m=2, c=C)
                nc.vector.tensor_tensor(
                    out=stg[:, ts(b, 4 * C)].rearrange("p (t c) -> p t c", c=C),
                    in0=t2v[:, :, 0, :],
                    in1=t2v[:, :, 1, :],
                    op=mybir.AluOpType.add,
                )

            gate = min(PE_DELAY_CHUNKS, n_chunks - 1)
            nc.tensor.matmul(
                scr, lhsT=xt[gate][:, 0, 0:P], rhs=wh_sb[:, 0, :],
                start=True, stop=True,
            )

            for g in range(n_chunks):
                for bi in range(batches_per_chunk):
                    # logits for this 4-tile batch, bias-seeded
                    lg = lg_pool.tile([P, 4 * MC], F32, name="lg")
                    nc.tensor.matmul(
                        lg, lhsT=ones2, rhs=bc4_sb, start=True, stop=False
                    )
                    for j in range(4):
                        t = bi * 4 + j
                        for k in range(2):
                            xh_c = xt[g][:, k, ts(t, P)]
                            xl_c = xt[g][:, 2 + k, ts(t, P)]
                            o = lg[:, ts(j, MC)]
                            nc.tensor.matmul(
                                o, lhsT=xh_c, rhs=wh_sb[:, k, :],
                                start=False, stop=False,
                            )
                            nc.tensor.matmul(
                                o, lhsT=xh_c, rhs=wl_sb[:, k, :],
                                start=False, stop=False,
                            )
                            nc.tensor.matmul(
                                o, lhsT=xl_c, rhs=wh_sb[:, k, :],
                                start=False, stop=(j == 3 and k == 1),
                            )
                    votes(g * batches_per_chunk + bi, lg)

            # single store at the end: the sync ring is idle once loads drain
            nc.sync.dma_start(ys[:], stg[:])
    nc.compile()
    return nc


_NC_CACHE: dict[int, bass.Bass] = {}


def _get_nc(b_shard: int) -> bass.Bass:
    if b_shard not in _NC_CACHE:
        _NC_CACHE[b_shard] = build_nc(b_shard)
    return _NC_CACHE[b_shard]


def _prep_inputs(x: np.ndarray, W: np.ndarray, b: np.ndarray):
    xf = np.asarray(x, dtype=np.float32)
    xh = xf.astype(np.float16)
    xl = (xf - xh.astype(np.float32)).astype(np.float16)
    # [B, 256] -> [128 d', 2 k, B] with d = 128k + d'
    parts = {
        "xh": xh.T.reshape(2, P, B_FULL).transpose(1, 0, 2),
        "xl": xl.T.reshape(2, P, B_FULL).transpose(1, 0, 2),
    }
    # m-major columns: col index = 10*m + c
    wf = np.asarray(W, dtype=np.float32).transpose(1, 0, 2).reshape(D, MC)
    whf = wf.astype(np.float16)
    wlf = (wf - whf.astype(np.float32)).astype(np.float16)
    bf = np.asarray(b, dtype=np.float32).reshape(MC)
    bh = bf.astype(np.float16)
    bl = (bf - bh.astype(np.float32)).astype(np.float16)
    bc4 = np.ascontiguousarray(
        np.stack([np.tile(bh, 4), np.tile(bl, 4)], axis=0)
    ).astype(np.float16)
    return parts, np.ascontiguousarray(whf), np.ascontiguousarray(wlf), bc4


def _unscramble(ys: np.ndarray) -> np.ndarray:
    # ys[p, b*40 + t*C + c] -> votes[b*512 + t*128 + p, c]
    n_batches = ys.shape[1] // (4 * C)
    return np.ascontiguousarray(
        ys.astype(np.float32)
        .reshape(P, n_batches, 4, C)
        .transpose(1, 2, 0, 3)
        .reshape(n_batches * 4 * P, C)
    )


def kernel(x: np.ndarray, W: np.ndarray, b: np.ndarray, **_) -> np.ndarray:
    from concourse.bass_utils import run_bass_kernel_spmd

    assert x.shape == (B_FULL, D), x.shape
    parts, whf, wlf, bc4 = _prep_inputs(x, W, b)

    nc = _get_nc(B_SHARD)
    in_maps = [
        {
            **{
                k: np.ascontiguousarray(v[:, :, i * B_SHARD : (i + 1) * B_SHARD])
                for k, v in parts.items()
            },
            "wh": whf,
            "wl": wlf,
            "bc4": bc4,
        }
        for i in range(N_CORES)
    ]
    res = run_bass_kernel_spmd(nc, in_maps, core_ids=list(range(N_CORES)))
    return np.concatenate(
        [_unscramble(res.results[i]["ys"]) for i in range(N_CORES)], axis=0
    )
